# revision 11
# baseline (speedup 1.0000x reference)
"""Multi-head self-attention 2d (B=2, C=256, H=W=64, 8 heads x 32 dim) on 8 TRN2 cores.

Sharding: batch (2-way) x query-rows-of-N=H*W (4-way) => 8 cores, no collectives.
v8: head-group-split projections + front-loaded quad 0 --
  - K and V projections are split by head group: quad 0 only computes/copies
    the hg0 halves it reads (1-bank PSUM slots, [128,512] copies); the hg1
    halves are projected during quad 1's own m-loop, halving quad-0's DVE
    copy load and balancing quads 0 and 1,
  - quad-0 projection matmuls are front-loaded to m=1..17 so the PE never
    idles long enough to trip the HAM re-throttle while the pipeline fills,
  - ScalarE stays pure-exp (all projection copies on DVE; head/tail exp bias
    keeps DVE free at quad ends), per-quad exp split [20,19,17,17],
  - at quad ends (qi<3) the two Op banks are evacuated by two DVE copies
    emitted before the next quad's first exp (banks free ~1.4us after the
    last AV, no HAM trip); the denominator row-shift runs on GpSimd, the
    aligned reciprocal on DVE, and the normalize multiplies as two
    full-partition [128,512] GpSimd tensor_tensors (persistent DN/RJ tiles
    with memset junk rows keep NaN out of the zero-padded proj rows),
  - outproj(qh=0) is deferred into quad 2 (ct=0 at m==8, ct=1 at m==12),
  - the last quad normalizes PSUM-direct on DVE interleaved with the output
    projection,
  - prologue: warm-up matmuls ramp the PE clock during the DMA wait; wq |
    xb-512-cols | wk DMA order; kproj-first emission with split K and Q
    copies and the first DVE exp hoisted before the deferred copy halves,
  - normalized attention outputs stay in the natural PSUM row layout with
    zero-padded host-side projection weights; xb rotated per-core on the
    host; V stored as [V_h | ones32] so AV emits denominators pre-broadcast.
"""

import os
import sys

import numpy as np

for _p in ("/opt/trn_rl_repo", "/root/.axon_site/_ro/trn_rl_repo"):
    if os.path.isdir(_p) and _p not in sys.path:
        sys.path.insert(0, _p)

import ml_dtypes
import concourse.bacc as bacc
import concourse.bass as bass
import concourse.tile as tile
from concourse import mybir
from concourse.bass_utils import run_bass_kernel_spmd

BF16 = mybir.dt.bfloat16
F32 = mybir.dt.float32
I16 = mybir.dt.int16
NPBF16 = ml_dtypes.bfloat16

NH, D = 8, 32          # heads, head dim
C = 256                # channels
N = 4096               # H*W positions
Q = 1024               # query shard per core
SCALE = 1.0 / np.sqrt(D)

# Schraudolph bf16 exp on the vector engine: bf16_bits(exp(y)) ~= round(y*128/ln2
# + (127*128 - c)). Fold the attention scale into the multiplier. c tuned for
# min max-rel-error under round-to-nearest (~3.3%).
SCH_A = float(SCALE * 128.0 / np.log(2.0))
SCH_B = float(127.0 * 128.0 - 5.5)
# Per-quad fraction (num/32) of exp tiles on ScalarE; quads 0/1 are ACT-heavy
# because DVE also drains their interleaved projection copies.
ACT_UNITS = [20, 19, 17, 17]
TAIL_M = 31            # m >= TAIL_M goes all-ACT (DVE runs the evac there)
TAIL_UNITS = 32
HEAD_M = 4             # first HEAD_M m-iters of quads 1-3 lean ACT (+HEAD_BUMP)
HEAD_BUMP = 7
WARM_MM = 14           # prologue warm-up matmuls (pstate + HAM ramp); must
                       # comfortably exceed the 3.4us HAM busy window


def _build_program():
    nc = bacc.Bacc("TRN2", target_bir_lowering=False, debug=False)

    xb = nc.dram_tensor("xb", [C, N], BF16, kind="ExternalInput")
    xq = nc.dram_tensor("xq", [C, Q], F32, kind="ExternalInput")
    wall = nc.dram_tensor("wall", [128, 2560], BF16, kind="ExternalInput")
    gam = nc.dram_tensor("gam", [128, 1], F32, kind="ExternalInput")
    out = nc.dram_tensor("out", [C, Q], F32, kind="ExternalOutput")

    with tile.TileContext(nc) as tc:
        _emit(tc, xb, xq, wall, gam, out)
    nc.compile()
    return nc


def _emit(tc, xb, xq, wall, gam, out):
    from contextlib import ExitStack

    nc = tc.nc
    Exp = mybir.ActivationFunctionType.Exp

    with ExitStack() as ctx:
        per = ctx.enter_context(tc.tile_pool(name="persist", bufs=1))

        def ptile(name, shape, dtype):
            return per.tile(shape, dtype, name=name, tag=name)

        XB = [ptile(f"XB{i}", [128, N], BF16) for i in range(2)]
        XQ = [ptile(f"XQ{i}", [128, Q], F32) for i in range(2)]
        W = ptile("W", [128, 2560], BF16)   # wq01 wk01 wv01 pj[hg=0,j=0..1] pj[hg=1,...]
        WQ = [W[:, 256 * c:256 * (c + 1)] for c in range(2)]
        WK = [W[:, 512 + 256 * c:512 + 256 * (c + 1)] for c in range(2)]
        WV = [W[:, 1024 + 256 * c:1024 + 256 * (c + 1)] for c in range(2)]
        PJP = [W[:, 1536 + 256 * g:1536 + 256 * (g + 1)] for g in range(4)]  # (2hg+j)
        G = ptile("G", [128, 1], F32)
        SCR = ptile("SCR", [128, 640], BF16)   # warm-up scratch
        Ksb = ptile("Ksb", [128, 2 * N], BF16)      # [dim-in-group, hg*4096 + key]
        Qsb = ptile("Qsb", [128, 2 * Q], BF16)      # [dim-in-group, hg*1024 + q]
        Vsb = ptile("Vsb", [128, 32 * 512], BF16)   # per m-chunk: 8 x [V(32)|ones(32)]
        # normalized attention outputs, natural PSUM row layout per (hg, j):
        # rows 0-31 head 2j, 32-63 junk, 64-95 head 2j+1, 96-127 junk
        OsbR = [ptile(f"OsbR{i}", [128, 2 * Q], BF16) for i in range(2)]
        # persistent staging for the quad-end normalize; junk rows memset once
        DN = ptile("DN", [128, 1024], F32)
        RJ = ptile("RJ", [128, 1024], F32)

        # DMAs spread across engine DGE queues so the transfers run in
        # parallel. Critical-path order: wq | xb first-512 | wk | xb rest.
        nc.scalar.dma_start(W[:, 0:512], wall[:, 0:512])          # wq
        nc.sync.dma_start(XB[0][:, 0:512], xb[0:128, 0:512])
        nc.gpsimd.dma_start(XB[1][:, 0:512], xb[128:256, 0:512])
        nc.scalar.dma_start(W[:, 512:1024], wall[:, 512:1024])    # wk
        nc.sync.dma_start(XB[0][:, 512:1024], xb[0:128, 512:1024])
        nc.gpsimd.dma_start(XB[1][:, 512:1024], xb[128:256, 512:1024])
        nc.scalar.dma_start(W[:, 1024:2560], wall[:, 1024:2560])  # wv + proj
        nc.sync.dma_start(XB[0][:, 1024:4096], xb[0:128, 1024:4096])
        nc.gpsimd.dma_start(XB[1][:, 1024:4096], xb[128:256, 1024:4096])
        nc.sync.dma_start(XQ[0][:], xq[0:128, :])
        nc.sync.dma_start(XQ[1][:], xq[128:256, :])
        nc.scalar.dma_start(G[:], gam[:, :])

        # one-time SBUF init on the idle GpSimd engine (behind its xb DMA
        # triggers): ones blocks of Vsb, the junk rows of OsbR (so 0-padded
        # proj rows never hit NaN*0), and the DN/RJ junk rows (DN=1 so the
        # reciprocal stays finite, RJ=0 so junk rows normalize to 0).
        v4 = Vsb.rearrange("p (mh w) -> p mh w", w=64)
        for m in range(32):
            nc.gpsimd.memset(v4[:, 8 * m:8 * (m + 1), 32:64], 1.0)
        for i in range(2):
            nc.gpsimd.memset(OsbR[i][32:64, :], 0.0)
            nc.gpsimd.memset(OsbR[i][96:128, :], 0.0)
        nc.gpsimd.memset(DN[32:64, :], 1.0)
        nc.gpsimd.memset(RJ[96:128, :], 0.0)

        exp_idx = [0]

        with ExitStack() as actx:
            sp = actx.enter_context(tc.tile_pool(name="sp", bufs=3, space="PSUM"))
            opl = actx.enter_context(tc.tile_pool(name="opl", bufs=1, space="PSUM"))
            pb = actx.enter_context(tc.tile_pool(name="pb", bufs=8))
            osb = actx.enter_context(tc.tile_pool(name="osb", bufs=2))
            rb = actx.enter_context(tc.tile_pool(name="rb", bufs=2))
            ob = actx.enter_context(tc.tile_pool(name="ob", bufs=2))

            def slot(name):
                return sp.tile([128, 1024], F32, name=name, tag="st2")

            def hslot(name):
                # 1-bank half slot for the hg-split projections
                return sp.tile([128, 512], F32, name=name, tag="st2")

            # PE warm-up: ramp pstate/HAM during the DMA wait. DVE memsets
            # the scratch; the matmuls chew on it into a transient PSUM slot.
            nc.vector.memset(SCR[:], 1.0)
            wp = slot("warm")
            for _ in range(WARM_MM):
                nc.tensor.matmul(wp[:, 0:512], lhsT=SCR[:, 0:128],
                                 rhs=SCR[:, 128:640], start=True, stop=True)

            k3 = Ksb.rearrange("p (h w) -> p h w", w=N)
            v5d = Vsb.rearrange("p (m h w) -> p m h w", h=8, w=64)

            def emit_qproj(p, split=False):
                qp = slot(f"qp{p}")
                for t2 in range(2):
                    ts_ = slice(512 * t2, 512 * (t2 + 1))
                    for c in range(2):
                        nc.tensor.matmul(qp[:, ts_], lhsT=WQ[c][:, 128 * p:128 * (p + 1)],
                                         rhs=XB[c][:, ts_], start=(c == 0), stop=(c == 1))
                if split:
                    nc.vector.tensor_copy(Qsb[:, 0:512], qp[:, 0:512])
                    return lambda: nc.vector.tensor_copy(Qsb[:, 512:1024], qp[:, 512:1024])
                nc.vector.tensor_copy(Qsb[:, 1024 * p:1024 * (p + 1)], qp[:])
                return None

            def emit_kproj(t, p, split=False):
                # K projection for head-group p, keys 512t..512t+512
                kp = hslot(f"kp{t}_{p}")
                xs = slice(512 * t, 512 * (t + 1))
                for c in range(2):
                    nc.tensor.matmul(kp[:], lhsT=WK[c][:, 128 * p:128 * (p + 1)],
                                     rhs=XB[c][:, xs], start=(c == 0), stop=(c == 1))
                if split:
                    # fast path for chunk 0: the first 128 keys land first so
                    # the first score can start immediately
                    nc.vector.tensor_copy(k3[:, p:p + 1, 0:128], kp[:, 0:128])
                    return lambda: nc.vector.tensor_copy(
                        k3[:, p:p + 1, 128:512], kp[:, 128:512])
                nc.vector.tensor_copy(k3[:, p:p + 1, 512 * t:512 * (t + 1)], kp[:])
                return None

            def emit_vproj(mq, hg):
                # V projection for head-group hg, chunks 4mq..4mq+3
                vp = hslot(f"vp{mq}_{hg}")
                for k in range(4):
                    m = 4 * mq + k
                    ms = slice(128 * m, 128 * (m + 1))
                    vs = slice(128 * k, 128 * (k + 1))
                    nc.tensor.matmul(vp[:, vs], lhsT=XB[0][:, ms],
                                     rhs=WV[0][:, 128 * hg:128 * (hg + 1)],
                                     start=True, stop=False)
                    nc.tensor.matmul(vp[:, vs], lhsT=XB[1][:, ms],
                                     rhs=WV[1][:, 128 * hg:128 * (hg + 1)],
                                     start=False, stop=True)
                nc.vector.tensor_copy(
                    v5d[:, 4 * mq:4 * (mq + 1), 4 * hg:4 * (hg + 1), 0:32],
                    vp[:].rearrange("p (k h d) -> p k h d", h=4, d=32))

            # flattened quad sequence: (qh, hg)
            quads = [(0, 0), (0, 1), (1, 0), (1, 1)]
            pts_by = {}
            Op_by = {}

            def emit_s(qi, m):
                qh, hg = quads[qi]
                units = ACT_UNITS[qi]
                if m >= TAIL_M:
                    u = TAIL_UNITS
                elif m < HEAD_M and qi > 0:
                    u = min(32, units + HEAD_BUMP)
                else:
                    u = units
                sts = [slot("st2s") for _ in range(2)]
                for g in range(2):
                    for j in range(2):
                        a = 2 * g + j
                        hh = slice(32 * a, 32 * (a + 1))
                        nc.tensor.matmul(
                            sts[g][:, 512 * j:512 * (j + 1)],
                            lhsT=Ksb[hh, N * hg + 128 * m:N * hg + 128 * (m + 1)],
                            rhs=Qsb[hh, Q * hg + 512 * qh:Q * hg + 512 * (qh + 1)],
                            start=True, stop=True,
                            tile_position=(32 * a, 0))
                pts = []
                for g in range(2):
                    pt2 = pb.tile([128, 1024], BF16, name="pt2", tag="pt2")
                    pts.append(pt2)
                    # accumulator Bresenham: even ACT/DVE interleave even as
                    # the ratio u changes across head/tail/quad transitions
                    exp_idx[0] += u
                    if exp_idx[0] >= 32:
                        exp_idx[0] -= 32
                        nc.scalar.activation(pt2[:], sts[g][:], Exp, scale=SCALE)
                    else:
                        nc.vector.tensor_scalar(
                            pt2.bitcast(I16)[:], sts[g][:], SCH_A, SCH_B,
                            mybir.AluOpType.mult, mybir.AluOpType.add)
                pts_by[(qi, m)] = pts

            def emit_av(qi, m):
                qh, hg = quads[qi]
                if m == 0:
                    Op_by[qi] = [opl.tile([128, 512], F32, name=f"Op{j}", tag=f"Op{j}")
                                 for j in range(2)]
                Op = Op_by[qi]
                pts = pts_by.pop((qi, m))
                first, last = m == 0, m == 31
                for j in range(2):
                    for b in range(2):
                        a = 2 * j + b
                        H = 4 * hg + a
                        nc.tensor.matmul(
                            Op[j][64 * b:64 * (b + 1), :],
                            lhsT=Vsb[:, 512 * m + 64 * H:512 * m + 64 * (H + 1)],
                            rhs=pts[j][:, 512 * b:512 * (b + 1)],
                            start=first, stop=last,
                            tile_position=(0, 64 * b), skip_group_check=True)

            def emit_evac(qi):
                # evacuate the Op banks with two DVE copies (DVE is idle --
                # the m=31 tail ran all-ACT); banks free ~1.4us after AV(31)
                Op = Op_by.pop(qi)
                OS = osb.tile([128, 1024], F32, name="OS", tag="OS")
                nc.vector.tensor_copy(OS[:, 0:512], Op[0][:])
                nc.vector.tensor_copy(OS[:, 512:1024], Op[1][:])
                return OS

            def emit_norm_staged(qi, OS):
                # off the critical path: shift the denominator rows down 32
                # on GpSimd, aligned reciprocal on DVE, then two
                # full-partition multiplies on GpSimd (junk rows multiply
                # DN/RJ memset values -- finite, and the zero-padded proj
                # weights ignore them).
                qh, hg = quads[qi]
                for b in range(2):
                    nc.vector.tensor_copy(DN[64 * b:64 * b + 32, :],
                                          OS[64 * b + 32:64 * b + 64, :])
                # single base-0 op: reciprocal_approx_fast misreads at
                # partition base 64 (rows 32-63 read the memset 1.0s)
                nc.vector.reciprocal_approx_fast(out=RJ[0:96, :], in_=DN[0:96, :])
                for j in range(2):
                    nc.gpsimd.tensor_tensor(
                        OsbR[hg][:, Q * j + 512 * qh:Q * j + 512 * (qh + 1)],
                        OS[:, 512 * j:512 * j + 512],
                        RJ[:, 512 * j:512 * j + 512],
                        mybir.AluOpType.mult)

            def emit_norm_tail_j(qi, j):
                # last quad: direct from PSUM on DVE (shortest latency)
                qh, hg = quads[qi]
                Op = Op_by[qi]
                rj = rb.tile([128, 512], F32, name="rjt", tag="rjt")
                nc.vector.reciprocal_approx_fast(out=rj[:, :], in_=Op[j][:])
                for b in range(2):
                    nc.vector.tensor_tensor(
                        OsbR[hg][64 * b:64 * b + 32,
                                 Q * j + 512 * qh:Q * j + 512 * (qh + 1)],
                        Op[j][64 * b:64 * b + 32, :],
                        rj[64 * b + 32:64 * b + 64, :],
                        mybir.AluOpType.mult)

            def emit_outproj_ct(qh, ct):
                qs = slice(512 * qh, 512 * (qh + 1))
                cs = slice(128 * ct, 128 * (ct + 1))
                pp2 = slot(f"op{ct}")
                for g in range(4):          # g = 2*hg + j
                    hg, j = divmod(g, 2)
                    nc.tensor.matmul(
                        pp2[:, :512], lhsT=PJP[g][:, cs],
                        rhs=OsbR[hg][:, Q * j + 512 * qh:Q * j + 512 * (qh + 1)],
                        start=(g == 0), stop=(g == 3))
                obt = ob.tile([128, 512], F32, name="obt", tag="obt")
                nc.vector.scalar_tensor_tensor(
                    obt[:], pp2[:, :512], G[:], XQ[ct][:, qs],
                    mybir.AluOpType.mult, mybir.AluOpType.add)
                nc.sync.dma_start(out[cs, qs], obt[:])

            def emit_outproj_tail(qi):
                # qh=1 output projection interleaved with the tail normalize:
                # g=0,1 (hg=0, normalized a quad ago) issue first; g=2,3
                # wait only on their own j's fresh multiplies.
                qh = 1
                qs = slice(512 * qh, 512 * (qh + 1))
                pp2s = [slot(f"op{ct}") for ct in range(2)]
                for ct in range(2):
                    cs = slice(128 * ct, 128 * (ct + 1))
                    for g in range(2):
                        nc.tensor.matmul(
                            pp2s[ct][:, :512], lhsT=PJP[g][:, cs],
                            rhs=OsbR[0][:, Q * g + 512 * qh:Q * g + 512 * (qh + 1)],
                            start=(g == 0), stop=False)
                for j in range(2):
                    emit_norm_tail_j(qi, j)
                    g = 2 + j
                    for ct in range(2):
                        cs = slice(128 * ct, 128 * (ct + 1))
                        nc.tensor.matmul(
                            pp2s[ct][:, :512], lhsT=PJP[g][:, cs],
                            rhs=OsbR[1][:, Q * j + 512 * qh:Q * j + 512 * (qh + 1)],
                            start=False, stop=(g == 3))
                Op_by.pop(qi)
                for ct in range(2):
                    cs = slice(128 * ct, 128 * (ct + 1))
                    obt = ob.tile([128, 512], F32, name="obt", tag="obt")
                    nc.vector.scalar_tensor_tensor(
                        obt[:], pp2s[ct][:, :512], G[:], XQ[ct][:, qs],
                        mybir.AluOpType.mult, mybir.AluOpType.add)
                    nc.sync.dma_start(out[cs, qs], obt[:])

            # prologue: hg0 K chunk 0 and the qh=0 half of Q first (split
            # copies, with the first DVE exp hoisted between the halves).
            krest = emit_kproj(0, 0, split=True)
            qrest = emit_qproj(0, split=True)
            emit_s(0, 0)
            krest()
            qrest()
            emit_kproj(1, 0)
            emit_vproj(0, 0)
            for it in range(128):
                qi, m = divmod(it, 32)
                if qi == 0:
                    # front-loaded hg0 projections + early hg1 seeds
                    if m % 2 == 1 and 3 <= m <= 13:
                        emit_kproj((m - 3) // 2 + 2, 0)       # t=2..7
                    if m % 2 == 0 and 2 <= m <= 14:
                        emit_vproj(m // 2, 0)                 # mq=1..7
                    if m == 15:
                        emit_qproj(1)
                    if m == 17:
                        emit_kproj(0, 1)
                    if m == 19:
                        emit_vproj(0, 1)
                    if m == 21:
                        emit_kproj(1, 1)
                elif qi == 1:
                    # hg1 projections just-in-time inside quad 1
                    if m % 4 == 2 and m // 4 + 2 < 8:
                        emit_kproj(m // 4 + 2, 1)
                    if m % 4 == 1 and m + 3 < 32:
                        emit_vproj((m + 3) // 4, 1)
                elif qi == 2:
                    if m == 8:
                        emit_outproj_ct(0, 0)
                    if m == 12:
                        emit_outproj_ct(0, 1)
                if m == 31:
                    # the next quad's first scores are emitted after the DVE
                    # evacuation so the Op-bank copies sit at the head of the
                    # idle DVE FIFO
                    emit_av(qi, m)
                    if qi < 3:
                        OS = emit_evac(qi)
                        if it + 1 < 128:
                            emit_s(*divmod(it + 1, 32))
                        emit_norm_staged(qi, OS)
                    else:
                        emit_outproj_tail(qi)
                else:
                    if it + 1 < 128:
                        emit_s(*divmod(it + 1, 32))
                    emit_av(qi, m)


_NC = None


def _get_program():
    global _NC
    if _NC is None:
        _NC = _build_program()
    return _NC


def kernel(x, qkv_w, proj_w, gamma, _trace=False):
    """Full inputs in, full output out. Shards across 8 NeuronCores internally."""
    nc = _get_program()
    B = x.shape[0]
    xf = np.ascontiguousarray(x.reshape(B, C, N).astype(np.float32))
    xf_bf = xf.astype(NPBF16)

    wqT = qkv_w[0:256].T.astype(NPBF16)
    wkT = qkv_w[256:512].T.astype(NPBF16)
    wvT = qkv_w[512:768].T.astype(NPBF16)
    pjT = proj_w.T.astype(NPBF16)
    # zero-padded proj tiles in the natural PSUM row layout of OsbR: for
    # g = 2*hg + j: rows 0-31 = head (4hg+2j) dims, 64-95 = head (4hg+2j+1)
    pjp = np.zeros((4, 128, 256), dtype=NPBF16)
    for g in range(4):
        hg, j = divmod(g, 2)
        h0 = 4 * hg + 2 * j
        pjp[g][0:32] = pjT[32 * h0:32 * (h0 + 1)]
        pjp[g][64:96] = pjT[32 * (h0 + 1):32 * (h0 + 2)]
    wall = np.ascontiguousarray(np.concatenate(
        [wqT[0:128], wqT[128:256], wkT[0:128], wkT[128:256],
         wvT[0:128], wvT[128:256], pjp[0], pjp[1], pjp[2], pjp[3]], axis=1))
    gam = np.full((128, 1), np.float32(gamma.reshape(-1)[0]), dtype=np.float32)

    in_maps = []
    for core in range(8):
        b, qi = divmod(core, 4)
        qs = slice(Q * qi, Q * (qi + 1))
        # rotate keys so this core's query block sits at columns 0-1023; key
        # order is irrelevant to attention (softmax + sum over keys).
        xrot = np.roll(xf_bf[b], -Q * qi, axis=1) if qi else xf_bf[b]
        in_maps.append({
            "xb": np.ascontiguousarray(xrot),
            "xq": np.ascontiguousarray(xf[b][:, qs]),
            "wall": wall,
            "gam": gam,
        })

    res = run_bass_kernel_spmd(nc, in_maps, core_ids=list(range(8)), trace=_trace)

    outf = np.empty((B, C, N), dtype=np.float32)
    for core in range(8):
        b, qi = divmod(core, 4)
        outf[b][:, Q * qi:Q * (qi + 1)] = res.results[core]["out"]
    result = outf.reshape(x.shape)
    if _trace:
        return result, res
    return result


# revision 13
# speedup vs baseline: 1.0440x; 1.0440x over previous
"""Multi-head self-attention 2d (B=2, C=256, H=W=64, 8 heads x 32 dim) on 8 TRN2 cores.

Sharding: batch (2-way) x query-rows-of-N=H*W (4-way) => 8 cores, no collectives.
v8: head-group-split projections + front-loaded quad 0 --
  - K and V projections are split by head group: quad 0 only computes/copies
    the hg0 halves it reads (1-bank PSUM slots, [128,512] copies); the hg1
    halves are projected during quad 1's own m-loop, halving quad-0's DVE
    copy load and balancing quads 0 and 1,
  - quad-0 projection matmuls are front-loaded to m=1..17 so the PE never
    idles long enough to trip the HAM re-throttle while the pipeline fills,
  - ScalarE stays pure-exp (all projection copies on DVE; head/tail exp bias
    keeps DVE free at quad ends), per-quad exp split [20,19,17,17],
  - at quad ends (qi<3) the two Op banks are evacuated by two DVE copies
    emitted before the next quad's first exp (banks free ~1.4us after the
    last AV, no HAM trip); the denominator row-shift runs on GpSimd, the
    aligned reciprocal on DVE, and the normalize multiplies as two
    full-partition [128,512] GpSimd tensor_tensors (persistent DN/RJ tiles
    with memset junk rows keep NaN out of the zero-padded proj rows),
  - outproj(qh=0) is deferred into quad 2 (ct=0 at m==8, ct=1 at m==12),
  - the last quad normalizes PSUM-direct on DVE interleaved with the output
    projection,
  - prologue: warm-up matmuls ramp the PE clock during the DMA wait; wq |
    xb-512-cols | wk DMA order; kproj-first emission with split K and Q
    copies and the first DVE exp hoisted before the deferred copy halves,
  - normalized attention outputs stay in the natural PSUM row layout with
    zero-padded host-side projection weights; xb rotated per-core on the
    host; V stored as [V_h | ones32] so AV emits denominators pre-broadcast.
"""

import os
import sys

import numpy as np

for _p in ("/opt/trn_rl_repo", "/root/.axon_site/_ro/trn_rl_repo"):
    if os.path.isdir(_p) and _p not in sys.path:
        sys.path.insert(0, _p)

import ml_dtypes
import concourse.bacc as bacc
import concourse.bass as bass
import concourse.tile as tile
from concourse import mybir
from concourse.bass_utils import run_bass_kernel_spmd

BF16 = mybir.dt.bfloat16
F32 = mybir.dt.float32
I16 = mybir.dt.int16
NPBF16 = ml_dtypes.bfloat16

NH, D = 8, 32          # heads, head dim
C = 256                # channels
N = 4096               # H*W positions
Q = 1024               # query shard per core
SCALE = 1.0 / np.sqrt(D)

# Schraudolph bf16 exp on the vector engine: bf16_bits(exp(y)) ~= round(y*128/ln2
# + (127*128 - c)). Fold the attention scale into the multiplier. c tuned for
# min max-rel-error under round-to-nearest (~3.3%).
SCH_A = float(SCALE * 128.0 / np.log(2.0))
SCH_B = float(127.0 * 128.0 - 5.5)
# Per-quad fraction (num/32) of exp tiles on ScalarE; quads 0/1 are ACT-heavy
# because DVE also drains their interleaved projection copies.
ACT_UNITS = [20, 18, 17, 17]
TAIL_M = 99            # disabled: AV deferral makes the evac timing tolerant
TAIL_UNITS = 32
HEAD_M = 4             # first HEAD_M m-iters of quads 1-3 lean ACT (+HEAD_BUMP)
HEAD_BUMP = 7
WARM_MM = 14           # prologue warm-up matmuls (pstate + HAM ramp); must
                       # comfortably exceed the 3.4us HAM busy window


def _build_program():
    nc = bacc.Bacc("TRN2", target_bir_lowering=False, debug=False)

    xb = nc.dram_tensor("xb", [C, N], BF16, kind="ExternalInput")
    xq = nc.dram_tensor("xq", [C, Q], F32, kind="ExternalInput")
    wall = nc.dram_tensor("wall", [128, 2560], BF16, kind="ExternalInput")
    gam = nc.dram_tensor("gam", [128, 1], F32, kind="ExternalInput")
    out = nc.dram_tensor("out", [C, Q], F32, kind="ExternalOutput")

    with tile.TileContext(nc) as tc:
        _emit(tc, xb, xq, wall, gam, out)
    nc.compile()
    return nc


def _emit(tc, xb, xq, wall, gam, out):
    from contextlib import ExitStack

    nc = tc.nc
    Exp = mybir.ActivationFunctionType.Exp

    with ExitStack() as ctx:
        per = ctx.enter_context(tc.tile_pool(name="persist", bufs=1))

        def ptile(name, shape, dtype):
            return per.tile(shape, dtype, name=name, tag=name)

        XB = [ptile(f"XB{i}", [128, N], BF16) for i in range(2)]
        XQ = [ptile(f"XQ{i}", [128, Q], F32) for i in range(2)]
        W = ptile("W", [128, 2560], BF16)   # wq01 wk01 wv01 pj[hg=0,j=0..1] pj[hg=1,...]
        WQ = [W[:, 256 * c:256 * (c + 1)] for c in range(2)]
        WK = [W[:, 512 + 256 * c:512 + 256 * (c + 1)] for c in range(2)]
        WV = [W[:, 1024 + 256 * c:1024 + 256 * (c + 1)] for c in range(2)]
        PJP = [W[:, 1536 + 256 * g:1536 + 256 * (g + 1)] for g in range(4)]  # (2hg+j)
        G = ptile("G", [128, 1], F32)
        SCR = ptile("SCR", [128, 640], BF16)   # warm-up scratch
        Ksb = ptile("Ksb", [128, 2 * N], BF16)      # [dim-in-group, hg*4096 + key]
        Qsb = ptile("Qsb", [128, 2 * Q], BF16)      # [dim-in-group, hg*1024 + q]
        Vsb = ptile("Vsb", [128, 32 * 512], BF16)   # per m-chunk: 8 x [V(32)|ones(32)]
        # normalized attention outputs, natural PSUM row layout per (hg, j):
        # rows 0-31 head 2j, 32-63 junk, 64-95 head 2j+1, 96-127 junk
        OsbR = [ptile(f"OsbR{i}", [128, 2 * Q], BF16) for i in range(2)]
        # persistent staging for the quad-end normalize; junk rows memset once
        DN = ptile("DN", [128, 1024], F32)
        RJ = ptile("RJ", [128, 1024], F32)

        # DMAs spread across engine DGE queues so the transfers run in
        # parallel. Critical-path order: wq | xb first-512 | wk | xb rest.
        nc.scalar.dma_start(W[:, 0:512], wall[:, 0:512])          # wq
        nc.sync.dma_start(XB[0][:, 0:512], xb[0:128, 0:512])
        nc.gpsimd.dma_start(XB[1][:, 0:512], xb[128:256, 0:512])
        nc.scalar.dma_start(W[:, 512:1024], wall[:, 512:1024])    # wk
        nc.sync.dma_start(XB[0][:, 512:1024], xb[0:128, 512:1024])
        nc.gpsimd.dma_start(XB[1][:, 512:1024], xb[128:256, 512:1024])
        nc.scalar.dma_start(W[:, 1024:2560], wall[:, 1024:2560])  # wv + proj
        nc.sync.dma_start(XB[0][:, 1024:4096], xb[0:128, 1024:4096])
        nc.gpsimd.dma_start(XB[1][:, 1024:4096], xb[128:256, 1024:4096])
        nc.sync.dma_start(XQ[0][:], xq[0:128, :])
        nc.sync.dma_start(XQ[1][:], xq[128:256, :])
        nc.scalar.dma_start(G[:], gam[:, :])

        # one-time SBUF init on the idle GpSimd engine (behind its xb DMA
        # triggers): ones blocks of Vsb, the junk rows of OsbR (so 0-padded
        # proj rows never hit NaN*0), and the DN/RJ junk rows (DN=1 so the
        # reciprocal stays finite, RJ=0 so junk rows normalize to 0).
        v4 = Vsb.rearrange("p (mh w) -> p mh w", w=64)
        for m in range(32):
            nc.gpsimd.memset(v4[:, 8 * m:8 * (m + 1), 32:64], 1.0)
        for i in range(2):
            nc.gpsimd.memset(OsbR[i][32:64, :], 0.0)
            nc.gpsimd.memset(OsbR[i][96:128, :], 0.0)
        nc.gpsimd.memset(DN[32:64, :], 1.0)
        nc.gpsimd.memset(RJ[96:128, :], 0.0)

        exp_idx = [0]

        with ExitStack() as actx:
            sp = actx.enter_context(tc.tile_pool(name="sp", bufs=3, space="PSUM"))
            opl = actx.enter_context(tc.tile_pool(name="opl", bufs=1, space="PSUM"))
            pb = actx.enter_context(tc.tile_pool(name="pb", bufs=8))
            osb = actx.enter_context(tc.tile_pool(name="osb", bufs=2))
            rb = actx.enter_context(tc.tile_pool(name="rb", bufs=2))
            ob = actx.enter_context(tc.tile_pool(name="ob", bufs=2))

            def slot(name):
                return sp.tile([128, 1024], F32, name=name, tag="st2")

            def hslot(name):
                # 1-bank half slot for the hg-split projections
                return sp.tile([128, 512], F32, name=name, tag="st2")

            # PE warm-up: ramp pstate/HAM during the DMA wait. DVE memsets
            # the scratch; the matmuls chew on it into a transient PSUM slot.
            nc.vector.memset(SCR[:], 1.0)
            wp = slot("warm")
            for _ in range(WARM_MM):
                nc.tensor.matmul(wp[:, 0:512], lhsT=SCR[:, 0:128],
                                 rhs=SCR[:, 128:640], start=True, stop=True)

            k3 = Ksb.rearrange("p (h w) -> p h w", w=N)
            v5d = Vsb.rearrange("p (m h w) -> p m h w", h=8, w=64)

            def emit_qproj(p, split=False):
                qp = slot(f"qp{p}")
                for t2 in range(2):
                    ts_ = slice(512 * t2, 512 * (t2 + 1))
                    for c in range(2):
                        nc.tensor.matmul(qp[:, ts_], lhsT=WQ[c][:, 128 * p:128 * (p + 1)],
                                         rhs=XB[c][:, ts_], start=(c == 0), stop=(c == 1))
                if split:
                    nc.vector.tensor_copy(Qsb[:, 0:512], qp[:, 0:512])
                    return lambda: nc.vector.tensor_copy(Qsb[:, 512:1024], qp[:, 512:1024])
                nc.vector.tensor_copy(Qsb[:, 1024 * p:1024 * (p + 1)], qp[:])
                return None

            def emit_kproj(t, p, split=False):
                # K projection for head-group p, keys 512t..512t+512
                kp = hslot(f"kp{t}_{p}")
                xs = slice(512 * t, 512 * (t + 1))
                for c in range(2):
                    nc.tensor.matmul(kp[:], lhsT=WK[c][:, 128 * p:128 * (p + 1)],
                                     rhs=XB[c][:, xs], start=(c == 0), stop=(c == 1))
                if split:
                    # fast path for chunk 0: the first 128 keys land first so
                    # the first score can start immediately
                    nc.vector.tensor_copy(k3[:, p:p + 1, 0:128], kp[:, 0:128])
                    return lambda: nc.vector.tensor_copy(
                        k3[:, p:p + 1, 128:512], kp[:, 128:512])
                nc.vector.tensor_copy(k3[:, p:p + 1, 512 * t:512 * (t + 1)], kp[:])
                return None

            def emit_vproj(mq, hg):
                # V projection for head-group hg, chunks 4mq..4mq+3
                vp = hslot(f"vp{mq}_{hg}")
                for k in range(4):
                    m = 4 * mq + k
                    ms = slice(128 * m, 128 * (m + 1))
                    vs = slice(128 * k, 128 * (k + 1))
                    nc.tensor.matmul(vp[:, vs], lhsT=XB[0][:, ms],
                                     rhs=WV[0][:, 128 * hg:128 * (hg + 1)],
                                     start=True, stop=False)
                    nc.tensor.matmul(vp[:, vs], lhsT=XB[1][:, ms],
                                     rhs=WV[1][:, 128 * hg:128 * (hg + 1)],
                                     start=False, stop=True)
                nc.vector.tensor_copy(
                    v5d[:, 4 * mq:4 * (mq + 1), 4 * hg:4 * (hg + 1), 0:32],
                    vp[:].rearrange("p (k h d) -> p k h d", h=4, d=32))

            # flattened quad sequence: (qh, hg)
            quads = [(0, 0), (0, 1), (1, 0), (1, 1)]
            pts_by = {}
            Op_by = {}

            def emit_s(qi, m):
                qh, hg = quads[qi]
                units = ACT_UNITS[qi]
                if m >= TAIL_M:
                    u = TAIL_UNITS
                elif m < HEAD_M and qi > 0:
                    u = min(32, units + HEAD_BUMP)
                else:
                    u = units
                sts = [slot("st2s") for _ in range(2)]
                for g in range(2):
                    for j in range(2):
                        a = 2 * g + j
                        hh = slice(32 * a, 32 * (a + 1))
                        nc.tensor.matmul(
                            sts[g][:, 512 * j:512 * (j + 1)],
                            lhsT=Ksb[hh, N * hg + 128 * m:N * hg + 128 * (m + 1)],
                            rhs=Qsb[hh, Q * hg + 512 * qh:Q * hg + 512 * (qh + 1)],
                            start=True, stop=True,
                            tile_position=(32 * a, 0))
                pts = []
                for g in range(2):
                    pt2 = pb.tile([128, 1024], BF16, name="pt2", tag="pt2")
                    pts.append(pt2)
                    # accumulator Bresenham: even ACT/DVE interleave even as
                    # the ratio u changes across head/tail/quad transitions
                    exp_idx[0] += u
                    if exp_idx[0] >= 32:
                        exp_idx[0] -= 32
                        nc.scalar.activation(pt2[:], sts[g][:], Exp, scale=SCALE)
                    else:
                        nc.vector.tensor_scalar(
                            pt2.bitcast(I16)[:], sts[g][:], SCH_A, SCH_B,
                            mybir.AluOpType.mult, mybir.AluOpType.add)
                pts_by[(qi, m)] = pts

            def emit_av(qi, m):
                qh, hg = quads[qi]
                if m == 0:
                    Op_by[qi] = [opl.tile([128, 512], F32, name=f"Op{j}", tag=f"Op{j}")
                                 for j in range(2)]
                Op = Op_by[qi]
                pts = pts_by.pop((qi, m))
                first, last = m == 0, m == 31
                for j in range(2):
                    for b in range(2):
                        a = 2 * j + b
                        H = 4 * hg + a
                        nc.tensor.matmul(
                            Op[j][64 * b:64 * (b + 1), :],
                            lhsT=Vsb[:, 512 * m + 64 * H:512 * m + 64 * (H + 1)],
                            rhs=pts[j][:, 512 * b:512 * (b + 1)],
                            start=first, stop=last,
                            tile_position=(0, 64 * b), skip_group_check=True)

            def emit_evac(qi):
                # evacuate the Op banks with two DVE copies (DVE is idle --
                # the m=31 tail ran all-ACT); banks free ~1.4us after AV(31)
                Op = Op_by.pop(qi)
                OS = osb.tile([128, 1024], F32, name="OS", tag="OS")
                nc.vector.tensor_copy(OS[:, 0:512], Op[0][:])
                nc.vector.tensor_copy(OS[:, 512:1024], Op[1][:])
                return OS

            def emit_norm_staged(qi, OS):
                # off the critical path: shift the denominator rows down 32
                # on GpSimd, aligned reciprocal on DVE, then two
                # full-partition multiplies on GpSimd (junk rows multiply
                # DN/RJ memset values -- finite, and the zero-padded proj
                # weights ignore them).
                qh, hg = quads[qi]
                for b in range(2):
                    nc.vector.tensor_copy(DN[64 * b:64 * b + 32, :],
                                          OS[64 * b + 32:64 * b + 64, :])
                # single base-0 op: reciprocal_approx_fast misreads at
                # partition base 64 (rows 32-63 read the memset 1.0s)
                nc.vector.reciprocal_approx_fast(out=RJ[0:96, :], in_=DN[0:96, :])
                for j in range(2):
                    nc.gpsimd.tensor_tensor(
                        OsbR[hg][:, Q * j + 512 * qh:Q * j + 512 * (qh + 1)],
                        OS[:, 512 * j:512 * j + 512],
                        RJ[:, 512 * j:512 * j + 512],
                        mybir.AluOpType.mult)

            def emit_norm_tail_j(qi, j):
                # last quad: direct from PSUM on DVE (shortest latency)
                qh, hg = quads[qi]
                Op = Op_by[qi]
                rj = rb.tile([128, 512], F32, name="rjt", tag="rjt")
                nc.vector.reciprocal_approx_fast(out=rj[:, :], in_=Op[j][:])
                for b in range(2):
                    nc.vector.tensor_tensor(
                        OsbR[hg][64 * b:64 * b + 32,
                                 Q * j + 512 * qh:Q * j + 512 * (qh + 1)],
                        Op[j][64 * b:64 * b + 32, :],
                        rj[64 * b + 32:64 * b + 64, :],
                        mybir.AluOpType.mult)

            def emit_outproj_ct(qh, ct):
                qs = slice(512 * qh, 512 * (qh + 1))
                cs = slice(128 * ct, 128 * (ct + 1))
                pp2 = slot(f"op{ct}")
                for g in range(4):          # g = 2*hg + j
                    hg, j = divmod(g, 2)
                    nc.tensor.matmul(
                        pp2[:, :512], lhsT=PJP[g][:, cs],
                        rhs=OsbR[hg][:, Q * j + 512 * qh:Q * j + 512 * (qh + 1)],
                        start=(g == 0), stop=(g == 3))
                obt = ob.tile([128, 512], F32, name="obt", tag="obt")
                nc.vector.scalar_tensor_tensor(
                    obt[:], pp2[:, :512], G[:], XQ[ct][:, qs],
                    mybir.AluOpType.mult, mybir.AluOpType.add)
                nc.sync.dma_start(out[cs, qs], obt[:])

            def emit_outproj_tail(qi):
                # qh=1 output projection interleaved with the tail normalize:
                # g=0,1 (hg=0, normalized a quad ago) issue first; g=2,3
                # wait only on their own j's fresh multiplies.
                qh = 1
                qs = slice(512 * qh, 512 * (qh + 1))
                pp2s = [slot(f"op{ct}") for ct in range(2)]
                for ct in range(2):
                    cs = slice(128 * ct, 128 * (ct + 1))
                    for g in range(2):
                        nc.tensor.matmul(
                            pp2s[ct][:, :512], lhsT=PJP[g][:, cs],
                            rhs=OsbR[0][:, Q * g + 512 * qh:Q * g + 512 * (qh + 1)],
                            start=(g == 0), stop=False)
                for j in range(2):
                    emit_norm_tail_j(qi, j)
                    g = 2 + j
                    for ct in range(2):
                        cs = slice(128 * ct, 128 * (ct + 1))
                        nc.tensor.matmul(
                            pp2s[ct][:, :512], lhsT=PJP[g][:, cs],
                            rhs=OsbR[1][:, Q * j + 512 * qh:Q * j + 512 * (qh + 1)],
                            start=False, stop=(g == 3))
                Op_by.pop(qi)
                for ct in range(2):
                    cs = slice(128 * ct, 128 * (ct + 1))
                    obt = ob.tile([128, 512], F32, name="obt", tag="obt")
                    nc.vector.scalar_tensor_tensor(
                        obt[:], pp2s[ct][:, :512], G[:], XQ[ct][:, qs],
                        mybir.AluOpType.mult, mybir.AluOpType.add)
                    nc.sync.dma_start(out[cs, qs], obt[:])

            # prologue: hg0 K chunk 0 and the qh=0 half of Q first (split
            # copies, with the first DVE exp hoisted between the halves).
            krest = emit_kproj(0, 0, split=True)
            qrest = emit_qproj(0, split=True)
            emit_s(0, 0)
            krest()
            qrest()
            emit_kproj(1, 0)
            emit_vproj(0, 0)
            # main loop with one-iteration AV deferral: when an AV enters the
            # PE FIFO its exp inputs finished a full iteration ago, so it
            # never head-of-line-blocks the score matmuls behind it.
            pending_av = None
            for it in range(128):
                qi, m = divmod(it, 32)
                if pending_av is not None:
                    pqi, pm = pending_av
                    emit_av(pqi, pm)
                    if pm == 31:
                        OS = emit_evac(pqi)
                        emit_norm_staged(pqi, OS)
                if qi == 0:
                    # front-loaded hg0 projections + early hg1 seeds
                    if m % 2 == 1 and 3 <= m <= 13:
                        emit_kproj((m - 3) // 2 + 2, 0)       # t=2..7
                    if m % 2 == 0 and 2 <= m <= 14:
                        emit_vproj(m // 2, 0)                 # mq=1..7
                    if m == 15:
                        emit_qproj(1)
                    if m == 17:
                        emit_kproj(0, 1)
                    if m == 19:
                        emit_vproj(0, 1)
                    if m == 21:
                        emit_kproj(1, 1)
                elif qi == 1:
                    # hg1 projections just-in-time inside quad 1
                    if m % 4 == 2 and m // 4 + 2 < 8:
                        emit_kproj(m // 4 + 2, 1)
                    if m % 4 == 1 and m + 3 < 32:
                        emit_vproj((m + 3) // 4, 1)
                elif qi == 2:
                    if m == 8:
                        emit_outproj_ct(0, 0)
                    if m == 12:
                        emit_outproj_ct(0, 1)
                if it + 1 < 128:
                    emit_s(*divmod(it + 1, 32))
                pending_av = (qi, m)
            emit_av(3, 31)
            emit_outproj_tail(3)


_NC = None


def _get_program():
    global _NC
    if _NC is None:
        _NC = _build_program()
    return _NC


def kernel(x, qkv_w, proj_w, gamma, _trace=False):
    """Full inputs in, full output out. Shards across 8 NeuronCores internally."""
    nc = _get_program()
    B = x.shape[0]
    xf = np.ascontiguousarray(x.reshape(B, C, N).astype(np.float32))
    xf_bf = xf.astype(NPBF16)

    wqT = qkv_w[0:256].T.astype(NPBF16)
    wkT = qkv_w[256:512].T.astype(NPBF16)
    wvT = qkv_w[512:768].T.astype(NPBF16)
    pjT = proj_w.T.astype(NPBF16)
    # zero-padded proj tiles in the natural PSUM row layout of OsbR: for
    # g = 2*hg + j: rows 0-31 = head (4hg+2j) dims, 64-95 = head (4hg+2j+1)
    pjp = np.zeros((4, 128, 256), dtype=NPBF16)
    for g in range(4):
        hg, j = divmod(g, 2)
        h0 = 4 * hg + 2 * j
        pjp[g][0:32] = pjT[32 * h0:32 * (h0 + 1)]
        pjp[g][64:96] = pjT[32 * (h0 + 1):32 * (h0 + 2)]
    wall = np.ascontiguousarray(np.concatenate(
        [wqT[0:128], wqT[128:256], wkT[0:128], wkT[128:256],
         wvT[0:128], wvT[128:256], pjp[0], pjp[1], pjp[2], pjp[3]], axis=1))
    gam = np.full((128, 1), np.float32(gamma.reshape(-1)[0]), dtype=np.float32)

    in_maps = []
    for core in range(8):
        b, qi = divmod(core, 4)
        qs = slice(Q * qi, Q * (qi + 1))
        # rotate keys so this core's query block sits at columns 0-1023; key
        # order is irrelevant to attention (softmax + sum over keys).
        xrot = np.roll(xf_bf[b], -Q * qi, axis=1) if qi else xf_bf[b]
        in_maps.append({
            "xb": np.ascontiguousarray(xrot),
            "xq": np.ascontiguousarray(xf[b][:, qs]),
            "wall": wall,
            "gam": gam,
        })

    res = run_bass_kernel_spmd(nc, in_maps, core_ids=list(range(8)), trace=_trace)

    outf = np.empty((B, C, N), dtype=np.float32)
    for core in range(8):
        b, qi = divmod(core, 4)
        outf[b][:, Q * qi:Q * (qi + 1)] = res.results[core]["out"]
    result = outf.reshape(x.shape)
    if _trace:
        return result, res
    return result


# revision 14
# speedup vs baseline: 1.0558x; 1.0114x over previous
"""Multi-head self-attention 2d (B=2, C=256, H=W=64, 8 heads x 32 dim) on 8 TRN2 cores.

Sharding: batch (2-way) x query-rows-of-N=H*W (4-way) => 8 cores, no collectives.
v8: head-group-split projections + front-loaded quad 0 --
  - K and V projections are split by head group: quad 0 only computes/copies
    the hg0 halves it reads (1-bank PSUM slots, [128,512] copies); the hg1
    halves are projected during quad 1's own m-loop, halving quad-0's DVE
    copy load and balancing quads 0 and 1,
  - quad-0 projection matmuls are front-loaded to m=1..17 so the PE never
    idles long enough to trip the HAM re-throttle while the pipeline fills,
  - ScalarE stays pure-exp (all projection copies on DVE; head/tail exp bias
    keeps DVE free at quad ends), per-quad exp split [20,19,17,17],
  - at quad ends (qi<3) the two Op banks are evacuated by two DVE copies
    emitted before the next quad's first exp (banks free ~1.4us after the
    last AV, no HAM trip); the denominator row-shift runs on GpSimd, the
    aligned reciprocal on DVE, and the normalize multiplies as two
    full-partition [128,512] GpSimd tensor_tensors (persistent DN/RJ tiles
    with memset junk rows keep NaN out of the zero-padded proj rows),
  - outproj(qh=0) is deferred into quad 2 (ct=0 at m==8, ct=1 at m==12),
  - the last quad normalizes PSUM-direct on DVE interleaved with the output
    projection,
  - prologue: warm-up matmuls ramp the PE clock during the DMA wait; wq |
    xb-512-cols | wk DMA order; kproj-first emission with split K and Q
    copies and the first DVE exp hoisted before the deferred copy halves,
  - normalized attention outputs stay in the natural PSUM row layout with
    zero-padded host-side projection weights; xb rotated per-core on the
    host; V stored as [V_h | ones32] so AV emits denominators pre-broadcast.
"""

import os
import sys

import numpy as np

for _p in ("/opt/trn_rl_repo", "/root/.axon_site/_ro/trn_rl_repo"):
    if os.path.isdir(_p) and _p not in sys.path:
        sys.path.insert(0, _p)

import ml_dtypes
import concourse.bacc as bacc
import concourse.bass as bass
import concourse.tile as tile
from concourse import mybir
from concourse.bass_utils import run_bass_kernel_spmd

BF16 = mybir.dt.bfloat16
F32 = mybir.dt.float32
I16 = mybir.dt.int16
NPBF16 = ml_dtypes.bfloat16

NH, D = 8, 32          # heads, head dim
C = 256                # channels
N = 4096               # H*W positions
Q = 1024               # query shard per core
SCALE = 1.0 / np.sqrt(D)

# Schraudolph bf16 exp on the vector engine: bf16_bits(exp(y)) ~= round(y*128/ln2
# + (127*128 - c)). Fold the attention scale into the multiplier. c tuned for
# min max-rel-error under round-to-nearest (~3.3%).
SCH_A = float(SCALE * 128.0 / np.log(2.0))
SCH_B = float(127.0 * 128.0 - 5.5)
# Per-quad fraction (num/32) of exp tiles on ScalarE; quads 0/1 are ACT-heavy
# because DVE also drains their interleaved projection copies.
ACT_UNITS = [20, 20, 16, 16]
TAIL_M = 99            # disabled: AV deferral makes the evac timing tolerant
TAIL_UNITS = 32
HEAD_M = 4             # first HEAD_M m-iters of quads 1-3 lean ACT (+HEAD_BUMP)
HEAD_BUMP = 7
WARM_MM = 18           # prologue warm-up matmuls (pstate + HAM ramp); must
                       # comfortably exceed the 3.4us HAM busy window


def _build_program():
    nc = bacc.Bacc("TRN2", target_bir_lowering=False, debug=False)

    xb = nc.dram_tensor("xb", [C, N], BF16, kind="ExternalInput")
    xq = nc.dram_tensor("xq", [C, Q], F32, kind="ExternalInput")
    wall = nc.dram_tensor("wall", [128, 2560], BF16, kind="ExternalInput")
    gam = nc.dram_tensor("gam", [128, 1], F32, kind="ExternalInput")
    out = nc.dram_tensor("out", [C, Q], F32, kind="ExternalOutput")

    with tile.TileContext(nc) as tc:
        _emit(tc, xb, xq, wall, gam, out)
    nc.compile()
    return nc


def _emit(tc, xb, xq, wall, gam, out):
    from contextlib import ExitStack

    nc = tc.nc
    Exp = mybir.ActivationFunctionType.Exp

    with ExitStack() as ctx:
        per = ctx.enter_context(tc.tile_pool(name="persist", bufs=1))

        def ptile(name, shape, dtype):
            return per.tile(shape, dtype, name=name, tag=name)

        XB = [ptile(f"XB{i}", [128, N], BF16) for i in range(2)]
        XQ = [ptile(f"XQ{i}", [128, Q], F32) for i in range(2)]
        W = ptile("W", [128, 2560], BF16)   # wq01 wk01 wv01 pj[hg=0,j=0..1] pj[hg=1,...]
        WQ = [W[:, 256 * c:256 * (c + 1)] for c in range(2)]
        WK = [W[:, 512 + 256 * c:512 + 256 * (c + 1)] for c in range(2)]
        WV = [W[:, 1024 + 256 * c:1024 + 256 * (c + 1)] for c in range(2)]
        PJP = [W[:, 1536 + 256 * g:1536 + 256 * (g + 1)] for g in range(4)]  # (2hg+j)
        G = ptile("G", [128, 1], F32)
        SCR = ptile("SCR", [128, 640], BF16)   # warm-up scratch
        Ksb = ptile("Ksb", [128, 2 * N], BF16)      # [dim-in-group, hg*4096 + key]
        Qsb = ptile("Qsb", [128, 2 * Q], BF16)      # [dim-in-group, hg*1024 + q]
        Vsb = ptile("Vsb", [128, 32 * 512], BF16)   # per m-chunk: 8 x [V(32)|ones(32)]
        # normalized attention outputs, natural PSUM row layout per (hg, j):
        # rows 0-31 head 2j, 32-63 junk, 64-95 head 2j+1, 96-127 junk
        OsbR = [ptile(f"OsbR{i}", [128, 2 * Q], BF16) for i in range(2)]
        # persistent staging for the quad-end normalize; junk rows memset once
        DN = ptile("DN", [128, 1024], F32)
        RJ = ptile("RJ", [128, 1024], F32)

        # DMAs spread across engine DGE queues so the transfers run in
        # parallel. Critical-path order: wq | xb first-512 | wk | xb rest.
        nc.scalar.dma_start(W[:, 0:512], wall[:, 0:512])          # wq
        nc.sync.dma_start(XB[0][:, 0:512], xb[0:128, 0:512])
        nc.gpsimd.dma_start(XB[1][:, 0:512], xb[128:256, 0:512])
        nc.scalar.dma_start(W[:, 512:1024], wall[:, 512:1024])    # wk
        nc.sync.dma_start(XB[0][:, 512:1024], xb[0:128, 512:1024])
        nc.gpsimd.dma_start(XB[1][:, 512:1024], xb[128:256, 512:1024])
        nc.scalar.dma_start(W[:, 1024:2560], wall[:, 1024:2560])  # wv + proj
        nc.sync.dma_start(XB[0][:, 1024:4096], xb[0:128, 1024:4096])
        nc.gpsimd.dma_start(XB[1][:, 1024:4096], xb[128:256, 1024:4096])
        nc.sync.dma_start(XQ[0][:], xq[0:128, :])
        nc.sync.dma_start(XQ[1][:], xq[128:256, :])
        nc.scalar.dma_start(G[:], gam[:, :])

        # one-time SBUF init on the idle GpSimd engine (behind its xb DMA
        # triggers): ones blocks of Vsb, the junk rows of OsbR (so 0-padded
        # proj rows never hit NaN*0), and the DN/RJ junk rows (DN=1 so the
        # reciprocal stays finite, RJ=0 so junk rows normalize to 0).
        v4 = Vsb.rearrange("p (mh w) -> p mh w", w=64)
        for m in range(32):
            nc.gpsimd.memset(v4[:, 8 * m:8 * (m + 1), 32:64], 1.0)
        for i in range(2):
            nc.gpsimd.memset(OsbR[i][32:64, :], 0.0)
            nc.gpsimd.memset(OsbR[i][96:128, :], 0.0)
        nc.gpsimd.memset(DN[32:64, :], 1.0)
        nc.gpsimd.memset(RJ[96:128, :], 0.0)

        exp_idx = [0]

        with ExitStack() as actx:
            sp = actx.enter_context(tc.tile_pool(name="sp", bufs=3, space="PSUM"))
            opl = actx.enter_context(tc.tile_pool(name="opl", bufs=1, space="PSUM"))
            pb = actx.enter_context(tc.tile_pool(name="pb", bufs=8))
            osb = actx.enter_context(tc.tile_pool(name="osb", bufs=2))
            rb = actx.enter_context(tc.tile_pool(name="rb", bufs=2))
            ob = actx.enter_context(tc.tile_pool(name="ob", bufs=2))

            def slot(name):
                return sp.tile([128, 1024], F32, name=name, tag="st2")

            def hslot(name):
                # 1-bank half slot for the hg-split projections
                return sp.tile([128, 512], F32, name=name, tag="st2")

            # PE warm-up: ramp pstate/HAM during the DMA wait. DVE memsets
            # the scratch; the matmuls chew on it into a transient PSUM slot.
            nc.vector.memset(SCR[:], 1.0)
            wp = slot("warm")
            for _ in range(WARM_MM):
                nc.tensor.matmul(wp[:, 0:512], lhsT=SCR[:, 0:128],
                                 rhs=SCR[:, 128:640], start=True, stop=True)

            k3 = Ksb.rearrange("p (h w) -> p h w", w=N)
            v5d = Vsb.rearrange("p (m h w) -> p m h w", h=8, w=64)

            def emit_qproj(p, split=False):
                qp = slot(f"qp{p}")
                for t2 in range(2):
                    ts_ = slice(512 * t2, 512 * (t2 + 1))
                    for c in range(2):
                        nc.tensor.matmul(qp[:, ts_], lhsT=WQ[c][:, 128 * p:128 * (p + 1)],
                                         rhs=XB[c][:, ts_], start=(c == 0), stop=(c == 1))
                if split:
                    nc.vector.tensor_copy(Qsb[:, 0:512], qp[:, 0:512])
                    return lambda: nc.vector.tensor_copy(Qsb[:, 512:1024], qp[:, 512:1024])
                nc.vector.tensor_copy(Qsb[:, 1024 * p:1024 * (p + 1)], qp[:])
                return None

            def emit_kproj(t, p, split=False):
                # K projection for head-group p, keys 512t..512t+512
                kp = hslot(f"kp{t}_{p}")
                xs = slice(512 * t, 512 * (t + 1))
                for c in range(2):
                    nc.tensor.matmul(kp[:], lhsT=WK[c][:, 128 * p:128 * (p + 1)],
                                     rhs=XB[c][:, xs], start=(c == 0), stop=(c == 1))
                if split:
                    # fast path for chunk 0: the first 128 keys land first so
                    # the first score can start immediately
                    nc.vector.tensor_copy(k3[:, p:p + 1, 0:128], kp[:, 0:128])
                    return lambda: nc.vector.tensor_copy(
                        k3[:, p:p + 1, 128:512], kp[:, 128:512])
                nc.vector.tensor_copy(k3[:, p:p + 1, 512 * t:512 * (t + 1)], kp[:])
                return None

            def emit_vproj(mq, hg):
                # V projection for head-group hg, chunks 4mq..4mq+3
                vp = hslot(f"vp{mq}_{hg}")
                for k in range(4):
                    m = 4 * mq + k
                    ms = slice(128 * m, 128 * (m + 1))
                    vs = slice(128 * k, 128 * (k + 1))
                    nc.tensor.matmul(vp[:, vs], lhsT=XB[0][:, ms],
                                     rhs=WV[0][:, 128 * hg:128 * (hg + 1)],
                                     start=True, stop=False)
                    nc.tensor.matmul(vp[:, vs], lhsT=XB[1][:, ms],
                                     rhs=WV[1][:, 128 * hg:128 * (hg + 1)],
                                     start=False, stop=True)
                nc.vector.tensor_copy(
                    v5d[:, 4 * mq:4 * (mq + 1), 4 * hg:4 * (hg + 1), 0:32],
                    vp[:].rearrange("p (k h d) -> p k h d", h=4, d=32))

            # flattened quad sequence: (qh, hg)
            quads = [(0, 0), (0, 1), (1, 0), (1, 1)]
            pts_by = {}
            Op_by = {}

            def emit_s(qi, m):
                qh, hg = quads[qi]
                units = ACT_UNITS[qi]
                if m >= TAIL_M:
                    u = TAIL_UNITS
                elif m < HEAD_M and qi > 0:
                    u = min(32, units + HEAD_BUMP)
                else:
                    u = units
                sts = [slot("st2s") for _ in range(2)]
                for g in range(2):
                    for j in range(2):
                        a = 2 * g + j
                        hh = slice(32 * a, 32 * (a + 1))
                        nc.tensor.matmul(
                            sts[g][:, 512 * j:512 * (j + 1)],
                            lhsT=Ksb[hh, N * hg + 128 * m:N * hg + 128 * (m + 1)],
                            rhs=Qsb[hh, Q * hg + 512 * qh:Q * hg + 512 * (qh + 1)],
                            start=True, stop=True,
                            tile_position=(32 * a, 0))
                pts = []
                for g in range(2):
                    pt2 = pb.tile([128, 1024], BF16, name="pt2", tag="pt2")
                    pts.append(pt2)
                    # accumulator Bresenham: even ACT/DVE interleave even as
                    # the ratio u changes across head/tail/quad transitions
                    exp_idx[0] += u
                    if exp_idx[0] >= 32:
                        exp_idx[0] -= 32
                        nc.scalar.activation(pt2[:], sts[g][:], Exp, scale=SCALE)
                    else:
                        nc.vector.tensor_scalar(
                            pt2.bitcast(I16)[:], sts[g][:], SCH_A, SCH_B,
                            mybir.AluOpType.mult, mybir.AluOpType.add)
                pts_by[(qi, m)] = pts

            def emit_av(qi, m):
                qh, hg = quads[qi]
                if m == 0:
                    Op_by[qi] = [opl.tile([128, 512], F32, name=f"Op{j}", tag=f"Op{j}")
                                 for j in range(2)]
                Op = Op_by[qi]
                pts = pts_by.pop((qi, m))
                first, last = m == 0, m == 31
                for j in range(2):
                    for b in range(2):
                        a = 2 * j + b
                        H = 4 * hg + a
                        nc.tensor.matmul(
                            Op[j][64 * b:64 * (b + 1), :],
                            lhsT=Vsb[:, 512 * m + 64 * H:512 * m + 64 * (H + 1)],
                            rhs=pts[j][:, 512 * b:512 * (b + 1)],
                            start=first, stop=last,
                            tile_position=(0, 64 * b), skip_group_check=True)

            def emit_evac(qi):
                # evacuate the Op banks with two DVE copies (DVE is idle --
                # the m=31 tail ran all-ACT); banks free ~1.4us after AV(31)
                Op = Op_by.pop(qi)
                OS = osb.tile([128, 1024], F32, name="OS", tag="OS")
                nc.vector.tensor_copy(OS[:, 0:512], Op[0][:])
                nc.vector.tensor_copy(OS[:, 512:1024], Op[1][:])
                return OS

            def emit_norm_staged(qi, OS):
                # off the critical path: shift the denominator rows down 32
                # on GpSimd, aligned reciprocal on DVE, then two
                # full-partition multiplies on GpSimd (junk rows multiply
                # DN/RJ memset values -- finite, and the zero-padded proj
                # weights ignore them).
                qh, hg = quads[qi]
                for b in range(2):
                    nc.vector.tensor_copy(DN[64 * b:64 * b + 32, :],
                                          OS[64 * b + 32:64 * b + 64, :])
                # single base-0 op: reciprocal_approx_fast misreads at
                # partition base 64 (rows 32-63 read the memset 1.0s)
                nc.vector.reciprocal_approx_fast(out=RJ[0:96, :], in_=DN[0:96, :])
                for j in range(2):
                    nc.gpsimd.tensor_tensor(
                        OsbR[hg][:, Q * j + 512 * qh:Q * j + 512 * (qh + 1)],
                        OS[:, 512 * j:512 * j + 512],
                        RJ[:, 512 * j:512 * j + 512],
                        mybir.AluOpType.mult)

            def emit_norm_tail_j(qi, j):
                # last quad: direct from PSUM on DVE (shortest latency)
                qh, hg = quads[qi]
                Op = Op_by[qi]
                rj = rb.tile([128, 512], F32, name="rjt", tag="rjt")
                nc.vector.reciprocal_approx_fast(out=rj[:, :], in_=Op[j][:])
                for b in range(2):
                    nc.vector.tensor_tensor(
                        OsbR[hg][64 * b:64 * b + 32,
                                 Q * j + 512 * qh:Q * j + 512 * (qh + 1)],
                        Op[j][64 * b:64 * b + 32, :],
                        rj[64 * b + 32:64 * b + 64, :],
                        mybir.AluOpType.mult)

            def emit_outproj_ct(qh, ct):
                qs = slice(512 * qh, 512 * (qh + 1))
                cs = slice(128 * ct, 128 * (ct + 1))
                pp2 = slot(f"op{ct}")
                for g in range(4):          # g = 2*hg + j
                    hg, j = divmod(g, 2)
                    nc.tensor.matmul(
                        pp2[:, :512], lhsT=PJP[g][:, cs],
                        rhs=OsbR[hg][:, Q * j + 512 * qh:Q * j + 512 * (qh + 1)],
                        start=(g == 0), stop=(g == 3))
                obt = ob.tile([128, 512], F32, name="obt", tag="obt")
                nc.vector.scalar_tensor_tensor(
                    obt[:], pp2[:, :512], G[:], XQ[ct][:, qs],
                    mybir.AluOpType.mult, mybir.AluOpType.add)
                nc.sync.dma_start(out[cs, qs], obt[:])

            def emit_outproj_tail(qi):
                # qh=1 output projection interleaved with the tail normalize:
                # g=0,1 (hg=0, normalized a quad ago) issue first; g=2,3
                # wait only on their own j's fresh multiplies.
                qh = 1
                qs = slice(512 * qh, 512 * (qh + 1))
                pp2s = [slot(f"op{ct}") for ct in range(2)]
                for ct in range(2):
                    cs = slice(128 * ct, 128 * (ct + 1))
                    for g in range(2):
                        nc.tensor.matmul(
                            pp2s[ct][:, :512], lhsT=PJP[g][:, cs],
                            rhs=OsbR[0][:, Q * g + 512 * qh:Q * g + 512 * (qh + 1)],
                            start=(g == 0), stop=False)
                for j in range(2):
                    emit_norm_tail_j(qi, j)
                    g = 2 + j
                    for ct in range(2):
                        cs = slice(128 * ct, 128 * (ct + 1))
                        nc.tensor.matmul(
                            pp2s[ct][:, :512], lhsT=PJP[g][:, cs],
                            rhs=OsbR[1][:, Q * j + 512 * qh:Q * j + 512 * (qh + 1)],
                            start=False, stop=(g == 3))
                Op_by.pop(qi)
                for ct in range(2):
                    cs = slice(128 * ct, 128 * (ct + 1))
                    obt = ob.tile([128, 512], F32, name="obt", tag="obt")
                    nc.vector.scalar_tensor_tensor(
                        obt[:], pp2s[ct][:, :512], G[:], XQ[ct][:, qs],
                        mybir.AluOpType.mult, mybir.AluOpType.add)
                    nc.sync.dma_start(out[cs, qs], obt[:])

            # prologue: hg0 K chunk 0 and the qh=0 half of Q first (split
            # copies, with the first DVE exp hoisted between the halves).
            krest = emit_kproj(0, 0, split=True)
            qrest = emit_qproj(0, split=True)
            emit_s(0, 0)
            krest()
            qrest()
            emit_kproj(1, 0)
            emit_vproj(0, 0)
            # main loop with one-iteration AV deferral: when an AV enters the
            # PE FIFO its exp inputs finished a full iteration ago, so it
            # never head-of-line-blocks the score matmuls behind it.
            pending_av = None
            for it in range(128):
                qi, m = divmod(it, 32)
                if pending_av is not None:
                    pqi, pm = pending_av
                    emit_av(pqi, pm)
                    if pm == 31:
                        OS = emit_evac(pqi)
                        emit_norm_staged(pqi, OS)
                if qi == 0:
                    # front-loaded hg0 projections + early hg1 seeds
                    if m % 2 == 1 and 3 <= m <= 13:
                        emit_kproj((m - 3) // 2 + 2, 0)       # t=2..7
                    if m % 2 == 0 and 2 <= m <= 14:
                        emit_vproj(m // 2, 0)                 # mq=1..7
                    if m == 15:
                        emit_qproj(1)
                    if m == 17:
                        emit_kproj(0, 1)
                    if m == 19:
                        emit_vproj(0, 1)
                    if m == 21:
                        emit_kproj(1, 1)
                elif qi == 1:
                    # hg1 projections just-in-time inside quad 1
                    if m % 4 == 2 and m // 4 + 2 < 8:
                        emit_kproj(m // 4 + 2, 1)
                    if m % 4 == 1 and m + 3 < 32:
                        emit_vproj((m + 3) // 4, 1)
                elif qi == 2:
                    if m == 8:
                        emit_outproj_ct(0, 0)
                    if m == 12:
                        emit_outproj_ct(0, 1)
                if it + 1 < 128:
                    emit_s(*divmod(it + 1, 32))
                pending_av = (qi, m)
            emit_av(3, 31)
            emit_outproj_tail(3)


_NC = None


def _get_program():
    global _NC
    if _NC is None:
        _NC = _build_program()
    return _NC


def kernel(x, qkv_w, proj_w, gamma, _trace=False):
    """Full inputs in, full output out. Shards across 8 NeuronCores internally."""
    nc = _get_program()
    B = x.shape[0]
    xf = np.ascontiguousarray(x.reshape(B, C, N).astype(np.float32))
    xf_bf = xf.astype(NPBF16)

    wqT = qkv_w[0:256].T.astype(NPBF16)
    wkT = qkv_w[256:512].T.astype(NPBF16)
    wvT = qkv_w[512:768].T.astype(NPBF16)
    pjT = proj_w.T.astype(NPBF16)
    # zero-padded proj tiles in the natural PSUM row layout of OsbR: for
    # g = 2*hg + j: rows 0-31 = head (4hg+2j) dims, 64-95 = head (4hg+2j+1)
    pjp = np.zeros((4, 128, 256), dtype=NPBF16)
    for g in range(4):
        hg, j = divmod(g, 2)
        h0 = 4 * hg + 2 * j
        pjp[g][0:32] = pjT[32 * h0:32 * (h0 + 1)]
        pjp[g][64:96] = pjT[32 * (h0 + 1):32 * (h0 + 2)]
    wall = np.ascontiguousarray(np.concatenate(
        [wqT[0:128], wqT[128:256], wkT[0:128], wkT[128:256],
         wvT[0:128], wvT[128:256], pjp[0], pjp[1], pjp[2], pjp[3]], axis=1))
    gam = np.full((128, 1), np.float32(gamma.reshape(-1)[0]), dtype=np.float32)

    in_maps = []
    for core in range(8):
        b, qi = divmod(core, 4)
        qs = slice(Q * qi, Q * (qi + 1))
        # rotate keys so this core's query block sits at columns 0-1023; key
        # order is irrelevant to attention (softmax + sum over keys).
        xrot = np.roll(xf_bf[b], -Q * qi, axis=1) if qi else xf_bf[b]
        in_maps.append({
            "xb": np.ascontiguousarray(xrot),
            "xq": np.ascontiguousarray(xf[b][:, qs]),
            "wall": wall,
            "gam": gam,
        })

    res = run_bass_kernel_spmd(nc, in_maps, core_ids=list(range(8)), trace=_trace)

    outf = np.empty((B, C, N), dtype=np.float32)
    for core in range(8):
        b, qi = divmod(core, 4)
        outf[b][:, Q * qi:Q * (qi + 1)] = res.results[core]["out"]
    result = outf.reshape(x.shape)
    if _trace:
        return result, res
    return result
